# revision 2
# baseline (speedup 1.0000x reference)
"""Cross-attention layer v2 on 8 TRN2 cores, one batch element per core.

Design (vs baseline kernel.py):
  - scores: row-tiled K=64 matmul pairs (tile_position (0,0)/(64,0)) — two
    heads concurrently in the PE array, ~2x the padded-K=128 baseline.
  - Q/K/V/O projections + ctx: fp8e4 DoubleRow matmuls (contraction 256 per
    pass, ~2x fp16 rate). Weights host-scaled x8 (fp8 normal range), descaled
    in the PSUM evictions.
  - probs in fp8e5: ACT exp (scale 1/8, bias -4) for most units; a share of
    units on DVE via the uint8-saturating exp2 bit trick (y=u8(A*s+B) bitcast
    e5m2), balancing the two engines.
  - ctx accumulators [65,512] with mask-weighted den as the 65th row;
    deferred normalization: drain fp16, batch-reciprocal the den rows per
    head pair, DRAM-round-trip partition broadcast, normalize on GPSIMD.
  - LayerNorm without bn_stats: mean via an extra fp8 column in the O-proj
    (colval = 64/768 * colsum(Wo8)), E[x^2] via ACT Square with accum_out,
    residual+descale via scalar_tensor_tensor.
  - x16 residual scaling (hsf = 16*(hs+bo+Wo@bv)) so ctx*16/den fits fp8;
    LN is scale-invariant with eps' = 256*eps.
"""

import numpy as np
import ml_dtypes
from contextlib import ExitStack

P = 128
H = 768
NH = 12
HD = 64
HT = H // P            # 6 feature tiles = head pairs
SHIFT = 4.0            # exp(s/8 - SHIFT)
A_TRICK = 4 * np.log2(np.e)          # e5m2 exp2-trick slope (per log-e arg)
E4 = ml_dtypes.float8_e4m3
E5 = ml_dtypes.float8_e5m2

DVE_SET = (1, 3, 5)    # units per (t,c) whose exp runs on DVE (uint8 trick)
DRAIN_ACT_MOD = 4      # (2t+c) % MOD == 0 -> ctx drain on ACT else DVE


def _bcast(ap, n, bass):
    return bass.AP(
        tensor=ap.tensor,
        offset=ap.offset,
        ap=[[0, n]] + [list(d) for d in ap.ap[1:]],
    )


def build_nc(SQ=1024, SK=1024, repeat=1, parts="all"):
    import concourse.bass as bass
    import concourse.bacc as bacc
    import concourse.tile as tile
    from concourse import mybir

    f32 = mybir.dt.float32
    f16 = mybir.dt.float16
    f8e4 = mybir.dt.float8e4
    f8e5 = mybir.dt.float8e5
    u8 = mybir.dt.uint8
    Alu = mybir.AluOpType
    Act = mybir.ActivationFunctionType
    DR = mybir.MatmulPerfMode.DoubleRow

    SQT = SQ // P      # 8
    SKT = SK // P      # 8
    NC2 = SQ // 512    # 2 query chunks
    WO_W = H + 16      # wo8 free width (mean col at 768, zero pad after)

    nc = bacc.Bacc(trn_type="TRN2", debug=False)

    hsT8 = nc.dram_tensor("hsT8", (P, HT, SQ), f8e4, kind="ExternalInput")
    cdT8 = nc.dram_tensor("cdT8", (P, HT, SK), f8e4, kind="ExternalInput")
    wq8 = nc.dram_tensor("wq8", (P, HT, H), f8e4, kind="ExternalInput")
    wk8 = nc.dram_tensor("wk8", (P, HT, H), f8e4, kind="ExternalInput")
    wv8 = nc.dram_tensor("wv8", (P, HT, H), f8e4, kind="ExternalInput")
    wo8 = nc.dram_tensor("wo8", (P, HT, WO_W), f8e4, kind="ExternalInput")
    bqv = nc.dram_tensor("bqv", (P, HT), f32, kind="ExternalInput")
    bkv = nc.dram_tensor("bkv", (P, HT), f32, kind="ExternalInput")
    wvs = nc.dram_tensor("wvs", (P, SKT), f32, kind="ExternalInput")
    wm12 = nc.dram_tensor("wm12", (P, SKT, NH), f8e4, kind="ExternalInput")
    hsf = nc.dram_tensor("hsf", (SQ, H), f16, kind="ExternalInput")
    hsfm = nc.dram_tensor("hsfm", (P, SQT), f32, kind="ExternalInput")
    lng = nc.dram_tensor("lng", (1, H), f16, kind="ExternalInput")
    lnb = nc.dram_tensor("lnb", (1, H), f16, kind="ExternalInput")
    out = nc.dram_tensor("out", (SQ, H), f16, kind="ExternalOutput")
    scr = nc.dram_tensor("scr", (HT, 2 * NC2, 512), f16, kind="Internal")

    with tile.TileContext(nc) as tc, ExitStack() as ctx:
        const = ctx.enter_context(tc.tile_pool(name="const", bufs=1))
        pers = ctx.enter_context(tc.tile_pool(name="pers", bufs=1))
        work = ctx.enter_context(tc.tile_pool(name="work", bufs=2))
        ps = ctx.enter_context(tc.tile_pool(name="ps", bufs=1, space="PSUM"))

        # ---- constants (outside repeat loop) ----
        wq_sb = const.tile([P, HT, H], f8e4, tag="wq")
        wk_sb = const.tile([P, HT, H], f8e4, tag="wk")
        wv_sb = const.tile([P, HT, H], f8e4, tag="wv")
        wo_sb = const.tile([P, HT, WO_W], f8e4, tag="wo")
        nc.sync.dma_start(out=wq_sb, in_=wq8[:, :, :])
        nc.sync.dma_start(out=wk_sb, in_=wk8[:, :, :])
        nc.sync.dma_start(out=wv_sb, in_=wv8[:, :, :])
        nc.sync.dma_start(out=wo_sb, in_=wo8[:, :, :])
        bq_sb = const.tile([P, HT], f32, tag="bq")
        bk_sb = const.tile([P, HT], f32, tag="bk")
        nc.sync.dma_start(out=bq_sb, in_=bqv[:, :])
        nc.sync.dma_start(out=bk_sb, in_=bkv[:, :])
        wvs_sb = const.tile([P, SKT], f32, tag="wvs")
        nc.sync.dma_start(out=wvs_sb, in_=wvs[:, :])
        hsfm_sb = const.tile([P, SQT], f32, tag="hsfm")
        nc.sync.dma_start(out=hsfm_sb, in_=hsfm[:, :])
        g_sb = const.tile([P, H], f16, tag="g")
        b_sb = const.tile([P, H], f16, tag="b")
        nc.sync.dma_start(out=g_sb, in_=_bcast(lng.ap(), P, bass))
        nc.sync.dma_start(out=b_sb, in_=_bcast(lnb.ap(), P, bass))
        sh_t = const.tile([P, 1], f32, tag="sh")
        nc.vector.memset(sh_t, -SHIFT)
        eps_t = const.tile([P, 1], f32, tag="eps")
        nc.vector.memset(eps_t, 256e-5)
        # vaug: mask-weighted V (cols 0:64) + w/16 den col (col 64).
        # den col is constant across iterations -> written once here.
        vaug = const.tile([P, SKT, NH, 80], f8e4, tag="vaug")
        wm12_sb = const.tile([P, SKT, NH], f8e4, tag="wm12")
        nc.sync.dma_start(out=wm12_sb, in_=wm12[:, :, :])
        nc.vector.tensor_copy(out=vaug[:, :, :, 64], in_=wm12_sb[:, :, :])

        def body(_iv=None):
            hs_sb = pers.tile([P, HT, SQ], f8e4, tag="hsT")
            cd_sb = pers.tile([P, HT, SK], f8e4, tag="cdT")
            nc.sync.dma_start(out=hs_sb, in_=hsT8[:, :, :])
            nc.scalar.dma_start(out=cd_sb, in_=cdT8[:, :, :])

            qT = pers.tile([P, HT, SQ], f16, tag="qT")
            kT = pers.tile([P, HT, SK], f16, tag="kT")
            ctxT8 = pers.tile([P, HT, SQ], f8e4, tag="ctxT8")

            spn = [0]

            def sp_slot(name):
                # next buffer in the 3-deep score/proj PSUM rotation
                tag = f"SP{spn[0] % 3}"
                spn[0] += 1
                return ps.tile([P, 2, 512], f32, tag=tag, name=name)

            def proj_one(wsb, src, dst, bias, t, name):
                pp = sp_slot(name)
                for c in range(NC2):
                    for j in range(HT // 2):
                        nc.tensor.matmul(
                            pp[:, c, :],
                            lhsT=wsb[:, 2 * j:2 * j + 2, t * P:(t + 1) * P],
                            rhs=src[:, 2 * j:2 * j + 2,
                                    c * 512:(c + 1) * 512],
                            start=(j == 0), stop=(j == 2), perf_mode=DR)
                nc.scalar.activation(
                    out=dst[:, t, :], in_=pp[:, :, :], func=Act.Identity,
                    scale=0.125, bias=bias[:, t:t + 1])

            def proj_q(t):
                proj_one(wq_sb, hs_sb, qT, bq_sb, t, f"pq{t}")

            def proj_k(t):
                proj_one(wk_sb, cd_sb, kT, bk_sb, t, f"pk{t}")

            def proj_v(m):
                # V projection for key tile m -> vaug[:, m, :, 0:64]
                pp = sp_slot(f"pv{m}")
                for c in range(2):
                    for j in range(HT // 2):
                        nc.tensor.matmul(
                            pp[:, c, 0:384],
                            lhsT=cd_sb[:, 2 * j:2 * j + 2,
                                       m * P:(m + 1) * P],
                            rhs=wv_sb[:, 2 * j:2 * j + 2,
                                      c * 384:(c + 1) * 384],
                            start=(j == 0), stop=(j == 2), perf_mode=DR)
                for c in range(2):
                    nc.vector.tensor_scalar(
                        out=vaug[:, m, 6 * c:6 * c + 6, 0:64],
                        in0=pp[:, c, 0:384].rearrange(
                            "p (n d) -> p n d", n=6),
                        scalar1=wvs_sb[:, m:m + 1], scalar2=None,
                        op0=Alu.mult)

            proj_q(0)
            proj_k(0)
            proj_v(0)
            proj_v(1)

            # ---- attention ----
            # pending per-(t,c) state
            cu_tiles = {}
            cc_tiles = {}

            def scores_pair(t, c, m):
                sp = sp_slot(f"s_{t}_{c}_{m}")
                for h, (p0, p1) in ((0, (0, 64)), (1, (64, 128))):
                    nc.tensor.matmul(
                        sp[:, h, :],
                        lhsT=kT[p0:p1, t, m * P:(m + 1) * P],
                        rhs=qT[p0:p1, t, c * 512:(c + 1) * 512],
                        start=True, stop=True, tile_position=(p0, 0))
                return sp

            def emit_exp(t, c, m, sp, etile):
                # probs for unit (t,c,m) -> etile[:, :, m % 2, :] (e5m2)
                dst = etile[:, :, m % 2, :]
                if m in DVE_SET:
                    nc.vector.tensor_scalar(
                        out=dst.bitcast(u8), in0=sp[:, :, :],
                        scalar1=float(A_TRICK * 0.125),
                        scalar2=float(4 * (15 - 0.0434) - SHIFT * A_TRICK),
                        op0=Alu.mult, op1=Alu.add)
                else:
                    nc.scalar.activation(
                        out=dst, in_=sp[:, :, :], func=Act.Exp,
                        scale=0.125, bias=sh_t)

            def emit_ctx(t, c, mj, etile, first, last):
                cc = cc_tiles[(t, c)]
                for h in range(2):
                    nc.tensor.matmul(
                        cc[0:65, h, :],
                        lhsT=vaug[:, 2 * mj:2 * mj + 2, 2 * t + h, 0:65],
                        rhs=etile[:, h, :, :],
                        start=first, stop=last, perf_mode=DR)

            def emit_drain(t, c):
                cc = cc_tiles.pop((t, c))
                cu = work.tile([65, 2, 512], f16, tag="cu", bufs=4,
                               name=f"cu_{t}_{c}")
                if (2 * t + c) % DRAIN_ACT_MOD == 0:
                    nc.scalar.activation(out=cu, in_=cc[0:65, :, :],
                                         func=Act.Copy, scale=1.0)
                else:
                    nc.vector.tensor_copy(out=cu, in_=cc[0:65, :, :])
                cu_tiles[(t, c)] = cu
                # den rows -> dencol_t rows (2c, 2c+1)
                dc = dencol[t % 2]
                nc.sync.dma_start(out=dc[2 * c:2 * c + 2, :],
                                  in_=cu[64:65, :, :])

            def emit_den_recip(t):
                dc = dencol[t % 2]
                rd = work.tile([2 * NC2, 512], f16, tag="rd", bufs=2,
                               name=f"rd{t}")
                with nc.allow_low_precision(reason="1/den at fp16"):
                    nc.vector.reciprocal(out=rd, in_=dc[:, :])
                nc.sync.dma_start(out=scr[t, :, :], in_=rd)

            def emit_norm(t, c):
                cu = cu_tiles.pop((t, c))
                rba = work.tile([64, 2, 512], f16, tag="rba", bufs=4,
                                name=f"rba_{t}_{c}")
                src = scr[t, 2 * c:2 * c + 2, :]
                nc.scalar.dma_start(
                    out=rba,
                    in_=bass.AP(tensor=src.tensor, offset=src.offset,
                                ap=[[0, 64]] + [list(d) for d in src.ap]))
                eng = nc.vector if t >= HT - 2 else nc.gpsimd
                eng.tensor_tensor(
                    out=ctxT8[0:64, t, c * 512:(c + 1) * 512],
                    in0=cu[0:64, 0, :], in1=rba[:, 0, :], op=Alu.mult)
                cn = work.tile([64, 512], f8e4, tag="cn", bufs=4,
                               name=f"cn_{t}_{c}")
                eng.tensor_tensor(out=cn, in0=cu[0:64, 1, :],
                                  in1=rba[:, 1, :], op=Alu.mult)
                nc.sync.dma_start(
                    out=ctxT8[64:128, t, c * 512:(c + 1) * 512],
                    in_=cn)

            dencol = [work.tile([2 * NC2, 512], f16, tag=f"dc{i}", bufs=1,
                                name=f"dencol{i}") for i in range(2)]

            pending = []   # closures to flush early in the next block
            for t in range(HT):
                for c in range(NC2):
                    cc_tiles[(t, c)] = ps.tile(
                        [P, 2, 512], f32, tag="C0",
                        name=f"cc_{t}_{c}")
                    etiles = []
                    for m in range(SKT):
                        sp = scores_pair(t, c, m)
                        if m % 2 == 0:
                            et = work.tile([P, 2, 2, 512], f8e5, tag="E",
                                           bufs=4, name=f"e_{t}_{c}_{m//2}")
                            etiles.append(et)
                        emit_exp(t, c, m, sp, etiles[m // 2])
                        if m == 1:
                            # previous block's final ctx group + drain
                            for fn in pending:
                                fn()
                            pending = []
                        # V projections interleaved into the first chunk
                        if t == 0 and c == 0 and m + 2 < SKT:
                            proj_v(m + 2)
                        # next head pair's projections, one slot at a time
                        if c == 0 and t + 1 < HT:
                            if m == 3:
                                proj_q(t + 1)
                            elif m == 5:
                                proj_k(t + 1)
                        # trailing ctx groups (one pair behind)
                        if m >= 3 and m % 2 == 1:
                            mj = (m - 3) // 2
                            emit_ctx(t, c, mj, etiles[mj],
                                     first=(mj == 0), last=False)

                    def flush(t=t, c=c, ets=etiles):
                        emit_ctx(t, c, 3, ets[3], first=False, last=True)
                        emit_drain(t, c)
                        if c == NC2 - 1:
                            emit_den_recip(t)
                            if t >= 1:
                                for cc_ in range(NC2):
                                    emit_norm(t - 1, cc_)
                    pending.append(flush)
            for fn in pending:
                fn()
            for c in range(NC2):
                emit_norm(HT - 1, c)

            # ---- out-projection + residual + LayerNorm ----
            if parts == "attn":
                nc.sync.dma_start(out=out[0:P, :],
                                  in_=qT[:, 0, 0:768])
                return
            for q in range(SQT):
                op = sp_slot(f"op{q}")
                for (c, c0, cw) in ((0, 0, 512), (1, 512, WO_W - 512)):
                    for j in range(HT // 2):
                        nc.tensor.matmul(
                            op[:, c, 0:cw],
                            lhsT=ctxT8[:, 2 * j:2 * j + 2,
                                       q * P:(q + 1) * P],
                            rhs=wo_sb[:, 2 * j:2 * j + 2, c0:c0 + cw],
                            start=(j == 0), stop=(j == 2), perf_mode=DR)
                hs_t = work.tile([P, H], f16, tag="hs", bufs=3,
                                 name=f"hs{q}")
                nc.sync.dma_start(out=hs_t, in_=hsf[q * P:(q + 1) * P, :])
                x_t = work.tile([P, H], f16, tag="x", bufs=3, name=f"x{q}")
                nc.vector.scalar_tensor_tensor(
                    out=x_t[:, 0:512], in0=op[:, 0, 0:512], scalar=0.125,
                    in1=hs_t[:, 0:512], op0=Alu.mult, op1=Alu.add)
                nc.vector.scalar_tensor_tensor(
                    out=x_t[:, 512:768], in0=op[:, 1, 0:256], scalar=0.125,
                    in1=hs_t[:, 512:768], op0=Alu.mult, op1=Alu.add)
                # mean: psum[:,1,256] = 512*mean(out16); mu = hsfm + /512
                mu = work.tile([P, 1], f32, tag="mu", bufs=3, name=f"mu{q}")
                nc.vector.tensor_scalar(
                    out=mu, in0=op[:, 1, 256:257], scalar1=1.0 / 512,
                    scalar2=hsfm_sb[:, q:q + 1], op0=Alu.mult, op1=Alu.add)
                # E[x^2] via ACT Square accumulate
                xsq = work.tile([P, H], f16, tag="xsq", bufs=2,
                                name=f"xsq{q}")
                sq = work.tile([P, 1], f32, tag="sq", bufs=3, name=f"sq{q}")
                nc.scalar.activation(out=xsq, in_=x_t, func=Act.Square,
                                     accum_out=sq)
                musq = work.tile([P, 1], f32, tag="musq", bufs=3,
                                 name=f"musq{q}")
                nc.vector.tensor_tensor(out=musq, in0=mu, in1=mu,
                                        op=Alu.mult)
                var = work.tile([P, 1], f32, tag="var", bufs=3,
                                name=f"var{q}")
                nc.vector.tensor_scalar(
                    out=var, in0=sq, scalar1=1.0 / H, scalar2=musq,
                    op0=Alu.mult, op1=Alu.subtract)
                rstd = work.tile([P, 1], f32, tag="rstd", bufs=3,
                                 name=f"rstd{q}")
                nc.scalar.activation(out=rstd, in_=var, func=Act.Sqrt,
                                     bias=eps_t, scale=1.0)
                nc.vector.reciprocal(out=rstd, in_=rstd)
                xn = work.tile([P, H], f16, tag="xn", bufs=3, name=f"xn{q}")
                nc.vector.tensor_scalar(
                    out=xn, in0=x_t, scalar1=mu, scalar2=rstd,
                    op0=Alu.subtract, op1=Alu.mult)
                xg = work.tile([P, H], f16, tag="xg", bufs=3, name=f"xg{q}")
                nc.gpsimd.tensor_tensor(out=xg, in0=xn, in1=g_sb,
                                        op=Alu.mult)
                xf = work.tile([P, H], f16, tag="xf", bufs=3, name=f"xf{q}")
                nc.vector.tensor_tensor(out=xf, in0=xg, in1=b_sb,
                                        op=Alu.add)
                nc.scalar.dma_start(out=out[q * P:(q + 1) * P, :], in_=xf)

        if repeat == 1:
            body()
        else:
            with tc.For_i(0, repeat,
                          hint_engines=(mybir.EngineType.PE,)) as iv:
                body(iv)

    nc.compile()
    return nc


def _q8(x):
    return np.clip(x, -240, 240).astype(E4)


def prep_core_inputs(hs_b, cd_b, mask_b, Wq, bq, Wk, bk, Wv, bv, Wo, bo,
                     ln_g, ln_b):
    f16 = np.float16
    f32 = np.float32
    SQ = hs_b.shape[0]
    SK = cd_b.shape[0]
    SKT = SK // P
    SQT = SQ // P
    WO_W = H + 16
    w = np.exp(-10000.0 * (1.0 - mask_b.astype(f32))).astype(f32)  # [SK]
    bo2 = bo.astype(f32) + Wo.astype(f32) @ bv.astype(f32)
    hsf = 16.0 * (np.ascontiguousarray(hs_b).astype(f32) + bo2[None, :])
    wcol = np.ascontiguousarray(w.reshape(SKT, P).T)

    def packT(W):   # [H,H] -> [P, HT, H] feature-major lhsT layout (x8 fp8)
        WT = np.ascontiguousarray(W.T.astype(f32) * 8.0)  # [in H, out H]
        return _q8(WT.reshape(HT, P, H).transpose(1, 0, 2))

    wo_pad = np.zeros((H, WO_W), f32)
    wo_pad[:, 0:H] = Wo.T.astype(f32) * 8.0      # [in feat, out]
    wo_pad[:, H] = (64.0 / H) * (Wo.astype(f32).sum(axis=0) * 8.0)
    wo8 = _q8(wo_pad.reshape(HT, P, WO_W).transpose(1, 0, 2))

    return {
        "hsT8": _q8(np.ascontiguousarray(hs_b.T).astype(f32)
                    .reshape(HT, P, SQ).transpose(1, 0, 2)),
        "cdT8": _q8(np.ascontiguousarray(cd_b.T).astype(f32)
                    .reshape(HT, P, SK).transpose(1, 0, 2)),
        "wq8": packT(Wq), "wk8": packT(Wk), "wv8": packT(Wv), "wo8": wo8,
        "bqv": np.ascontiguousarray(bq.reshape(HT, P).T).astype(f32),
        "bkv": np.ascontiguousarray(bk.reshape(HT, P).T).astype(f32),
        "wvs": (wcol / 8.0).astype(f32),
        "wm12": _q8(np.repeat((wcol / 16.0)[:, :, None], NH, axis=2)),
        "hsf": hsf.astype(f16),
        "hsfm": np.ascontiguousarray(
            hsf.mean(axis=1).reshape(SQT, P).T).astype(f32),
        "lng": ln_g.reshape(1, H).astype(f16),
        "lnb": ln_b.reshape(1, H).astype(f16),
    }


_NC_CACHE = {}


def kernel(hidden_states, condition_embeddings, condition_mask,
           Wq, bq, Wk, bk, Wv, bv, Wo, bo, ln_g, ln_b):
    from concourse.bass_utils import run_bass_kernel_spmd

    args = [np.asarray(a) for a in
            (hidden_states, condition_embeddings, condition_mask,
             Wq, bq, Wk, bk, Wv, bv, Wo, bo, ln_g, ln_b)]
    (hs, cd, mask, Wq, bq, Wk, bk, Wv, bv, Wo, bo, ln_g, ln_b) = args
    B, SQ, _ = hs.shape
    SK = cd.shape[1]

    key = (SQ, SK)
    if key not in _NC_CACHE:
        _NC_CACHE[key] = build_nc(SQ=SQ, SK=SK)
    nc = _NC_CACHE[key]

    in_maps = [
        prep_core_inputs(hs[b], cd[b], mask[b], Wq, bq, Wk, bk, Wv, bv,
                         Wo, bo, ln_g, ln_b)
        for b in range(B)
    ]
    res = run_bass_kernel_spmd(nc, in_maps, core_ids=list(range(B)))
    return np.stack(
        [res.results[b]["out"].astype(np.float32) for b in range(B)], axis=0
    )
